# revision 5
# baseline (speedup 1.0000x reference)
"""CompressedIndicatorEmbedding kernel for 8 Trainium2 NeuronCores.

out[n] = sum_p W[:, p*512 + x[n, p]]  for x [N=1048576, 8] int32, W [64, 4096] f32.

Data-parallel over tokens (N/8 per core). Per core, per 1024-token block the
lookup is computed as 32 PSUM-accumulated fp16 matmuls per 128-token tile:
  psum[128 tok, 64] += OH_k[128 rows, 128 tok].T @ Wchunk_k[128 rows, 64]
with a grouped-segment contraction layout: chunk k's partition q = 16p + r
holds the indicator of (segment p = q//16, code value 16k + q%16).  All 8
segments share one chunk, so x only needs a 16-fold partition broadcast
(256B/token of DMA instead of 2KB/token), done in a single DMA per block.
The indicator is exact (codes < 2048 are exact in fp16), W is fp16
(rel err ~3e-4).

Overheads are kept off the critical path: one input DMA (SP queue) and one
output DMA (GPSIMD queue) per block, PSUM->SBUF copies on the Scalar engine,
and 4 blocks per hardware-loop iteration to amortize the loop-boundary
engine rendezvous.  The kernel is Vector-engine bound (32 is_equal ops of
[128, 1024] per block at DVE 4x throughput).
"""
import sys
sys.path.insert(0, "/opt/trn_rl_repo")
import numpy as np
import concourse.bacc as bacc
import concourse.bass as bass
import concourse.mybir as mybir
from concourse.tile import TileContext
from concourse.bass_utils import run_bass_kernel_spmd

N_CORES = 8
N = 1048576
P = 8
L = 512
D = 64
T = N // N_CORES          # tokens per core
BLK = 1024                # tokens per block
NB = T // BLK
SUB = BLK // 128
UNROLL = 4                # blocks per hardware-loop iteration
F32, F16 = mybir.dt.float32, mybir.dt.float16
PE = mybir.EngineType.PE

_CACHED_NC = None


def _build():
    nc = bacc.Bacc("TRN2", target_bir_lowering=False, debug=False,
                   enable_asserts=False, num_devices=1)
    xt = nc.dram_tensor("xt", [P, T], F16, kind="ExternalInput")
    wt = nc.dram_tensor("wt", [128, 32 * 64], F16, kind="ExternalInput")
    iot = nc.dram_tensor("iot", [128, 32], F32, kind="ExternalInput")
    out = nc.dram_tensor("out", [T, D], F32, kind="ExternalOutput")

    with TileContext(nc) as tc:
        with tc.tile_pool(name="const", bufs=1) as cpool, \
             tc.tile_pool(name="xrep", bufs=3) as rpool, \
             tc.tile_pool(name="oh", bufs=12) as opool, \
             tc.tile_pool(name="psum", bufs=1, space="PSUM") as ppool, \
             tc.tile_pool(name="osb", bufs=3) as spool:
            w = cpool.tile([128, 32 * 64], F16)
            nc.sync.dma_start(w[:], wt[:])
            io = cpool.tile([128, 32], F32)
            nc.sync.dma_start(io[:], iot[:])

            def body(i, j, last=False):
                xr = rpool.tile([128, BLK], F16, tag="xr", name="xr")
                nc.sync.dma_start(
                    xr[:],
                    xt[:, bass.ds(i * (UNROLL * BLK) + j * BLK, BLK)].rearrange(
                        "p (g t) -> p g t", g=1).to_broadcast([P, 16, BLK]))
                psums = [ppool.tile([128, 64], F32, tag=f"ps{s}", name=f"ps{s}")
                         for s in range(SUB)]
                for k in range(32):
                    oh = opool.tile([128, BLK], F16, tag="oh", name="oh")
                    nc.vector.tensor_scalar(
                        oh[:], xr[:], io[:, k:k + 1], None,
                        mybir.AluOpType.is_equal)
                    for s in range(SUB):
                        nc.tensor.matmul(
                            psums[s][:],
                            oh[:, s * 128:(s + 1) * 128],
                            w[:, k * 64:(k + 1) * 64],
                            start=(k == 0), stop=(k == 31))
                ot = spool.tile([128, SUB * 64], F32, tag="ot", name="ot")
                # last body per iteration: copy on the (otherwise idle) DVE to
                # shorten the loop-boundary critical tail (ACT copies + its
                # per-block table load sit on the rendezvous path)
                for s in range(SUB):
                    if last:
                        nc.vector.tensor_copy(
                            ot[:, s * 64:(s + 1) * 64], psums[s][:])
                    else:
                        nc.scalar.copy(ot[:, s * 64:(s + 1) * 64], psums[s][:])
                nc.gpsimd.dma_start(
                    out[bass.ds(i * (UNROLL * BLK) + j * BLK, BLK), :].rearrange(
                        "(s t) d -> t s d", s=SUB),
                    ot[:].rearrange("t (s d) -> t s d", s=SUB))

            with tc.For_i(0, NB // UNROLL, 1, hint_engines=(PE,),
                          staggered_reset=True) as i:
                for j in range(UNROLL):
                    body(i, j, last=(j == UNROLL - 1))
    nc.compile()
    return nc


def _pack_tables(W: np.ndarray):
    """Host-side layout of the weight table (grouped-segment chunks, fp16)."""
    Wt = np.ascontiguousarray(W.T).astype(np.float16)          # [4096, 64]
    q = np.arange(128)
    k = np.arange(32)
    codes = (512 * (q[:, None] // 16) + 16 * k[None, :] + (q[:, None] % 16))
    wt = np.ascontiguousarray(Wt[codes].reshape(128, 32 * 64))
    iot = (16.0 * k[None, :] + (q[:, None] % 16)).astype(np.float32)
    iot = np.ascontiguousarray(np.broadcast_to(iot, (128, 32)))
    return wt, iot


def prep_in_maps(x: np.ndarray, W: np.ndarray):
    wt, iot = _pack_tables(W)
    in_maps = []
    for c in range(N_CORES):
        xc = x[c * T:(c + 1) * T]                              # [T, 8] int32
        xtc = np.ascontiguousarray(xc.T).astype(np.float16)    # [8, T]
        in_maps.append({"xt": xtc, "wt": wt, "iot": iot})
    return in_maps


def kernel(x: np.ndarray, W: np.ndarray) -> np.ndarray:
    global _CACHED_NC
    assert x.shape == (N, P) and W.shape == (D, P * L)
    if _CACHED_NC is None:
        _CACHED_NC = _build()
    nc = _CACHED_NC
    in_maps = prep_in_maps(x, W)
    res = run_bass_kernel_spmd(nc, in_maps, core_ids=list(range(N_CORES)))
    return np.concatenate(
        [res.results[c]["out"] for c in range(N_CORES)], axis=0)
